# revision 11
# baseline (speedup 1.0000x reference)
"""GroupedQueryAttention (B=2, S=2048, D=2048, Hq=32, Hkv=8) on 8 trn2 cores.

Sharding: pure data-parallel, zero collectives. Core c handles batch c//4 and
query tokens [512*(c%4), 512*(c%4)+512). Each core computes K^T / V for all
2048 keys of its batch (4x duplicated K/V projection work, but no cross-core
communication or collectives anywhere).

Per-core layout (everything transposed so the systolic contraction lands on
partitions):
  Q^T [2048 qdim, 512 tok]    lhsT=Wq tiles, rhs=Xq^T tiles
  K^T [512 kvdim, 2048 tok]   lhsT=Wk tiles, rhs=Xk^T tiles
  V   [2048 tok, 512 dv]      lhsT=Xv^T tiles, rhs=Wv tiles
  S^T [sk, sq] per head       lhsT=K^T slice [64,128], rhs=Q^T slice [64,512]
                              (two heads packed in array rows 0-63 / 64-127)
  P   = exp(S^T/8) bf16       ScalarE, batched 4 psum banks per call
  O^T [65, 512] per head      lhsT=[V|ones] [128,65] -> row 64 = softmax rowsum
  out^T [2048 dout, 512 tok]  lhsT=Wo tiles, rhs=normalized attn tiles

Biases: bk is dropped (adds a per-query constant to scores; softmax is
invariant to row shifts - exact). bv is folded into bo on the host
(softmax rows sum to 1, so attn@(V+bv) = attn@V + bv - exact). bq/bo are
applied on-device as per-partition activation biases.
"""

import os
import sys
from contextlib import ExitStack

import numpy as np
import ml_dtypes

for _p in ("/opt/trn_rl_repo",):
    if os.path.isdir(_p) and _p not in sys.path:
        sys.path.insert(0, _p)

import concourse.bass as bass
import concourse.tile as tile
from concourse import bacc, mybir
from concourse.bass_utils import run_bass_kernel_spmd

BF16 = mybir.dt.bfloat16
F32 = mybir.dt.float32
AF = mybir.ActivationFunctionType

D = 2048        # d_model
S = 2048        # sequence length
B = 2           # batch
HQ, HKV, HD = 32, 8, 64
KV = HKV * HD   # 512
P = 128
NT = D // P     # 16 d_model tiles
NSK = S // P    # 16 key tiles
TOK = S // 4    # 512 query tokens per core
NCORES = 8
SCALE = 1.0 / float(np.sqrt(HD))
ts = bass.ts


def build_program():
    nc = bacc.Bacc(
        "TRN2", target_bir_lowering=False, debug=False, num_devices=NCORES
    )

    # DRAM I/O (host pre-tiles the big weight/activation matrices so every
    # DMA below reads a contiguous 32KB block).
    xq_t = nc.dram_tensor("xq_t", [NT, P, TOK], BF16, kind="ExternalInput").ap()
    xk_t = nc.dram_tensor("xk_t", [D, S], BF16, kind="ExternalInput").ap()
    xv_t = nc.dram_tensor("xv_t", [NT, NSK, P, P], BF16, kind="ExternalInput").ap()
    wq = nc.dram_tensor("wq", [NT, NT, P, P], BF16, kind="ExternalInput").ap()
    wk = nc.dram_tensor("wk", [NT, P, KV], BF16, kind="ExternalInput").ap()
    wv = nc.dram_tensor("wv", [NT, P, KV], BF16, kind="ExternalInput").ap()
    wo = nc.dram_tensor("wo", [NT, NT, P, P], BF16, kind="ExternalInput").ap()
    bq_r = nc.dram_tensor("bq_r", [P, NT], F32, kind="ExternalInput").ap()
    bo_r = nc.dram_tensor("bo_r", [P, NT], F32, kind="ExternalInput").ap()
    out_t = nc.dram_tensor("out_t", [NT, P, TOK], F32, kind="ExternalOutput").ap()

    with tile.TileContext(nc) as tc, ExitStack() as ctx:
        consts = ctx.enter_context(tc.tile_pool(name="consts", bufs=1))
        stream = ctx.enter_context(tc.tile_pool(name="stream", bufs=4))
        espool = ctx.enter_context(tc.tile_pool(name="es", bufs=3))
        nrmpool = ctx.enter_context(tc.tile_pool(name="nrm", bufs=2))
        drampool = ctx.enter_context(tc.tile_pool(name="dbounce", bufs=2, space="DRAM"))
        outpool = ctx.enter_context(tc.tile_pool(name="outp", bufs=3))
        # PSUM budget is 8 banks and pool allocation is static: 4-bank "quad"
        # shared by K-proj and attention scores, 2x 1-bank "acc" shared by the
        # Q/V/Wo accumulators, 2x 1-bank attention outputs.
        quadps = ctx.enter_context(tc.tile_pool(name="quadps", bufs=1, space="PSUM"))
        accps = ctx.enter_context(tc.tile_pool(name="accps", bufs=2, space="PSUM"))
        opsum = ctx.enter_context(tc.tile_pool(name="opsum", bufs=1, space="PSUM"))

        # ---- persistent SBUF tensors -------------------------------------
        bq_sb = consts.tile([P, NT], F32)
        nc.sync.dma_start(bq_sb[:], bq_r[:])
        bo_sb = consts.tile([P, NT], F32)
        nc.sync.dma_start(bo_sb[:], bo_r[:])
        wk_sb = consts.tile([P, NT, KV], BF16)
        wv_sb = consts.tile([P, NT, KV], BF16)
        for t in range(NT):
            nc.sync.dma_start(wk_sb[:, t, :], wk[t])
            nc.sync.dma_start(wv_sb[:, t, :], wv[t])
        xqt_sb = consts.tile([P, NT, TOK], BF16)
        for t in range(NT):
            nc.sync.dma_start(xqt_sb[:, t, :], xq_t[t])
        kt_sb = [
            consts.tile([P, S], BF16, tag=f"kt{k}", name=f"kt{k}") for k in range(4)
        ]
        v8_sb = consts.tile([P, NSK, HKV, HD + 1], BF16)
        qt_sb = consts.tile([P, NT, TOK], BF16)
        attn_sb = consts.tile([P, NT, TOK], BF16)

        # ones column (col 64 of every [V|ones] stationary tile)
        nc.vector.memset(v8_sb[:], 1.0)

        # ---- K^T projection: kt_sb[k] = (Xk Wk)^T slice ------------------
        for blk in range(4):
            kps = quadps.tile([P, 4, TOK], F32, tag="quad", name="kps")
            for t in range(NT):
                xk_tile = stream.tile([P, TOK], BF16, tag="xk")
                nc.sync.dma_start(xk_tile[:], xk_t[ts(t, P), ts(blk, TOK)])
                for k in range(4):
                    nc.tensor.matmul(
                        kps[:, k, :],
                        wk_sb[:, t, ts(k, P)],
                        xk_tile[:],
                        start=(t == 0),
                        stop=(t == NT - 1),
                    )
            for k in range(4):
                nc.vector.tensor_copy(kt_sb[k][:, ts(blk, TOK)], kps[:, k, :])

        # ---- Q^T projection ---------------------------------------------
        for m in range(NT):
            qps = accps.tile([P, TOK], F32, tag="acc", name="qps")
            for t in range(NT):
                wq_tile = stream.tile([P, P], BF16, tag="wq")
                nc.sync.dma_start(wq_tile[:], wq[t, m])
                nc.tensor.matmul(
                    qps[:], wq_tile[:], xqt_sb[:, t, :],
                    start=(t == 0), stop=(t == NT - 1),
                )
            nc.scalar.activation(
                qt_sb[:, m, :], qps[:], AF.Identity, bias=bq_sb[:, m : m + 1]
            )

        # ---- V projection (untransposed: [tok, dv]) ----------------------
        for m in range(NSK):
            vps = accps.tile([P, KV], F32, tag="acc", name="vps")
            for t in range(NT):
                xv_tile = stream.tile([P, P], BF16, tag="xv")
                nc.sync.dma_start(xv_tile[:], xv_t[t, m])
                nc.tensor.matmul(
                    vps[:], xv_tile[:], wv_sb[:, t, :],
                    start=(t == 0), stop=(t == NT - 1),
                )
            nc.vector.tensor_copy(
                v8_sb[:, m, :, 0:HD],
                vps[:].rearrange("p (j d) -> p j d", j=HKV),
            )

        # ---- attention: 16 head-pairs ------------------------------------
        for hp in range(NT):
            k4 = hp % 4                      # K^T tile holding both kv heads
            j1, j2 = (2 * hp) % HKV, (2 * hp + 1) % HKV
            o1 = opsum.tile([HD + 1, TOK], F32, tag="o1")
            o2 = opsum.tile([HD + 1, TOK], F32, tag="o2")
            for th in range(NSK // 2):
                t0, t1 = 2 * th, 2 * th + 1
                stp = quadps.tile([P, 4, TOK], F32, tag="quad", name="stp")
                # scores^T, two heads packed into array rows 0-63 / 64-127
                nc.tensor.matmul(
                    stp[:, 0, :], kt_sb[k4][0:HD, ts(t0, P)],
                    qt_sb[0:HD, hp, :], start=True, stop=True,
                )
                nc.tensor.matmul(
                    stp[:, 1, :], kt_sb[k4][0:HD, ts(t1, P)],
                    qt_sb[0:HD, hp, :], start=True, stop=True,
                )
                nc.tensor.matmul(
                    stp[:, 2, :], kt_sb[k4][HD:P, ts(t0, P)],
                    qt_sb[HD:P, hp, :], start=True, stop=True,
                )
                nc.tensor.matmul(
                    stp[:, 3, :], kt_sb[k4][HD:P, ts(t1, P)],
                    qt_sb[HD:P, hp, :], start=True, stop=True,
                )
                es = espool.tile([P, 4, TOK], BF16, tag="es")
                nc.scalar.activation(es[:], stp[:], AF.Exp, scale=SCALE)
                # PV accumulation ([V|ones] stationary -> row 64 is rowsum)
                nc.tensor.matmul(
                    o1[:], v8_sb[:, t0, j1, :], es[:, 0, :],
                    start=(th == 0), stop=False,
                )
                nc.tensor.matmul(
                    o1[:], v8_sb[:, t1, j1, :], es[:, 1, :],
                    start=False, stop=(th == NSK // 2 - 1),
                )
                nc.tensor.matmul(
                    o2[:], v8_sb[:, t0, j2, :], es[:, 2, :],
                    start=(th == 0), stop=False,
                )
                nc.tensor.matmul(
                    o2[:], v8_sb[:, t1, j2, :], es[:, 3, :],
                    start=False, stop=(th == NSK // 2 - 1),
                )
            # softmax normalization: multiply by 1/rowsum, broadcast across
            # partitions via a DRAM bounce (partition-stride-0 DMA read).
            # Copy psum->SBUF first so the o1/o2 banks free immediately.
            # partition-broadcast of 1/rowsum via log-doubling partition-shift
            # DMAs (plain SBUF->SBUF DMAs; broadcast-AP DMAs and gpsimd
            # partition_broadcast both fail on this runtime).
            o1sb = nrmpool.tile([HD + 1, TOK], F32, tag="o1sb")
            nc.vector.tensor_copy(o1sb[:], o1[:])
            rs1 = nrmpool.tile([P, TOK], F32, tag="rs1")
            nc.vector.reciprocal(rs1[HD : HD + 1, :], o1sb[HD : HD + 1, :])
            nc.sync.dma_start(rs1[0:1, :], rs1[HD : HD + 1, :])
            for sh in (1, 2, 4, 8, 16, 32):
                nc.sync.dma_start(rs1[sh : 2 * sh, :], rs1[0:sh, :])
            nc.vector.tensor_mul(attn_sb[0:HD, hp, :], o1sb[0:HD, :], rs1[0:HD, :])
            o2sb = nrmpool.tile([HD + 1, TOK], F32, tag="o2sb")
            nc.vector.tensor_copy(o2sb[:], o2[:])
            rs2 = nrmpool.tile([P, TOK], F32, tag="rs2")
            nc.vector.reciprocal(rs2[HD : HD + 1, :], o2sb[HD : HD + 1, :])
            nc.sync.dma_start(rs2[0:1, :], rs2[HD : HD + 1, :])
            for sh in (1, 2, 4, 8, 16, 32):
                nc.sync.dma_start(rs2[sh : 2 * sh, :], rs2[0:sh, :])
            on2 = nrmpool.tile([HD, TOK], BF16, tag="on2")
            nc.vector.tensor_mul(on2[:], o2sb[0:HD, :], rs2[0:HD, :])
            # DVE lanes are partition-locked; shift odd head to partitions
            # 64-127 of the attn tile with an SBUF->SBUF DMA.
            nc.sync.dma_start(attn_sb[HD:P, hp, :], on2[:])

        # ---- output projection: out^T = Wo^T @ attn^T --------------------
        for d in range(NT):
            ops = accps.tile([P, TOK], F32, tag="acc", name="wops")
            for a in range(NT):
                wo_tile = stream.tile([P, P], BF16, tag="wo")
                nc.sync.dma_start(wo_tile[:], wo[a, d])
                nc.tensor.matmul(
                    ops[:], wo_tile[:], attn_sb[:, a, :],
                    start=(a == 0), stop=(a == NT - 1),
                )
            osb = outpool.tile([P, TOK], F32, tag="osb")
            nc.scalar.activation(osb[:], ops[:], AF.Identity, bias=bo_sb[:, d : d + 1])
            nc.sync.dma_start(out_t[d], osb[:])

    nc.compile()
    return nc


_NC_CACHE = None


def _get_program():
    global _NC_CACHE
    if _NC_CACHE is None:
        _NC_CACHE = build_program()
    return _NC_CACHE


def make_in_maps(query, key, value, Wq, bq, Wk, bk, Wv, bv, Wo, bo):
    bf = ml_dtypes.bfloat16
    f32 = np.float32
    query, key, value = (np.asarray(a, f32) for a in (query, key, value))
    Wq, Wk, Wv, Wo = (np.asarray(a, f32) for a in (Wq, Wk, Wv, Wo))
    bq, bk, bv, bo = (np.asarray(a, f32) for a in (bq, bk, bv, bo))

    wq_t = np.ascontiguousarray(
        Wq.astype(bf).reshape(NT, P, NT, P).transpose(0, 2, 1, 3)
    )
    wk_a = np.ascontiguousarray(Wk.astype(bf).reshape(NT, P, KV))
    wv_a = np.ascontiguousarray(Wv.astype(bf).reshape(NT, P, KV))
    wo_t = np.ascontiguousarray(
        Wo.astype(bf).reshape(NT, P, NT, P).transpose(0, 2, 1, 3)
    )
    # bk: dropped (softmax row-shift invariant). bv: folded into bo.
    beta = np.tile(bv, HQ // HKV)
    bo_eff = (bo.astype(np.float64) + beta.astype(np.float64) @ Wo.astype(np.float64))
    bq_r = np.ascontiguousarray(bq.reshape(NT, P).T)
    bo_r = np.ascontiguousarray(bo_eff.astype(f32).reshape(NT, P).T)

    per_batch = []
    for b in range(B):
        xk = np.ascontiguousarray(key[b].T.astype(bf))
        xvt = value[b].T.astype(bf)  # [dm, tok]
        xv_tiled = np.ascontiguousarray(
            xvt.reshape(NT, P, NSK, P).transpose(0, 2, 1, 3)
        )
        xqt = query[b].T.astype(bf)  # [dm, tok-all]
        per_batch.append((xk, xv_tiled, xqt))

    in_maps = []
    for c in range(NCORES):
        b, gi = divmod(c, 4)
        xk, xv_tiled, xqt = per_batch[b]
        xq_c = np.ascontiguousarray(
            xqt[:, gi * TOK : (gi + 1) * TOK].reshape(NT, P, TOK)
        )
        in_maps.append(
            {
                "xq_t": xq_c,
                "xk_t": xk,
                "xv_t": xv_tiled,
                "wq": wq_t,
                "wk": wk_a,
                "wv": wv_a,
                "wo": wo_t,
                "bq_r": bq_r,
                "bo_r": bo_r,
            }
        )
    return in_maps


def assemble_output(results):
    out = np.empty((B, S, D), np.float32)
    for c in range(NCORES):
        b, gi = divmod(c, 4)
        ot = np.asarray(results[c]["out_t"]).reshape(D, TOK)
        out[b, gi * TOK : (gi + 1) * TOK, :] = ot.T
    return out


def kernel(query, key, value, Wq, bq, Wk, bk, Wv, bv, Wo, bo):
    nc = _get_program()
    in_maps = make_in_maps(query, key, value, Wq, bq, Wk, bk, Wv, bv, Wo, bo)
    res = run_bass_kernel_spmd(nc, in_maps, list(range(NCORES)))
    return assemble_output(res.results)
